# revision 26
# baseline (speedup 1.0000x reference)
"""Multi-head attention (B=4, N=2048, C=1024, H=16, HD=64) on 8 TRN2 NeuronCores.

Self-contained: takes the FULL unsharded inputs of the reference problem,
shards across 8 cores, runs a Bass/Tile kernel SPMD, and reassembles the
full output.

Sharding (tensor-parallel over heads x data-parallel over batch):
  core i -> batch b = i//2, head-group g = i%2 (8 of the 16 heads).
  Unsharding sums the two partial output projections per batch.

Inputs are cast to bf16 on the host; partial outputs return as bf16 and
are summed in f32 on host.

The kernel is one long software-pipelined stream built around the
attention score chunks. Between score chunks the emitter interleaves
"side tasks" on the PE: QKV-projection tiles (just before the head that
needs them), V-projection tiles, the PV (attention*V) accumulations of
the previous head with their normalize/transpose epilogues, and
output-projection tiles of the previous query block. PE is the
bottleneck engine (~measured-rate roofline); ScalarE streams exp
underneath it.

Score matmuls run with a FULL 128-deep contraction even though HD=64:
the K of a head PAIR is stacked on partitions (natural layout), and the
moving Q operand is zero-padded on the other head's 64 rows, so the
other head contributes exactly 0. (64-deep matmuls stream at half rate
on TRN2 hardware; this trick doubles score throughput at zero accuracy
cost.) The padded per-head Q tiles are built into a small ring by the
otherwise-idle Pool engine.

exp split: 14/16 key-chunks on ScalarE (exact, with the softmax
denominator accumulated via an extra ones-column in V), 2/16 on DVE via
a Schraudolph-style fast exp (single tensor_scalar: bf16 bits =
round(s*a+b) as int16; zero-mean calibrated, ~1.8% elementwise noise
that averages out in the PV reduction).

PSUM: a 3-deep ring of [128,1024]f32 slots shared by score chunks,
QKV/V-projection tiles and output-projection tiles (6 banks), plus a
2-deep ring of single-bank slots each holding a PV accumulator (f32
cols 0:65) and the PE-transpose staging area (bf16 cols 512:640).
"""

import sys

sys.path.insert(0, "/opt/trn_rl_repo")

from contextlib import ExitStack

import numpy as np
import ml_dtypes

import concourse.bass as bass
import concourse.tile as tile
from concourse import mybir
from concourse.masks import make_identity
from concourse.bass_utils import run_bass_kernel_spmd

F32 = mybir.dt.float32
BF16 = mybir.dt.bfloat16
I16 = mybir.dt.int16

B, N, C, H = 4, 2048, 1024, 16
HH = H // 2          # heads per core
HD = C // H          # head dim
DH = HH * HD         # attention channels per core
OUTC = C
CO = C // 128
DO = DH // 128
NO128 = N // 128
NO512 = N // 512
KO = N // 128
QB = 1024
NQB = N // QB
QBO = QB // 128
NU = NQB * HH        # pipeline units
SCALE = float(HD) ** -0.5

LOG2E = 1.4426950408889634
SCH_C = 0.0581        # zero-mean Schraudolph bias (round-to-nearest convert)
SCH_A = SCALE * 128.0 * LOG2E
SCH_B = (127.0 - SCH_C) * 128.0
DVE_KOS = (5, 13)     # 2/16 key-chunks fast-exp on DVE: keeps ScalarE
                      # clearly below PE so the psS ring never throttles


def _split_multi_waits(nc, max_waits=1):
    """The pinned walrus build rejects >1 SyncWait on engine CTRL
    instructions; move extra waits onto preceding NOPs."""
    n_split = 0
    for bb in nc.main_func.blocks:
        insts = list(bb.instructions)
        new_insts = []
        changed = False
        for ins in insts:
            si = getattr(ins, "sync_info", None)
            nm = type(ins).__name__
            is_dma = "Dma" in nm or "TensorLoad" in nm or "TensorSave" in nm
            if si is not None and not is_dma:
                waits = list(si.on_wait)
                if len(waits) > max_waits:
                    head, tail = waits[:-max_waits], waits[-max_waits:]
                    for i in range(0, len(head), max_waits):
                        new_insts.append(
                            mybir.InstNoOp(
                                name=f"{ins.name}-ws{n_split}-{i}",
                                engine=ins.engine,
                                sync_info=mybir.SyncInfo(
                                    on_wait=head[i : i + max_waits], on_update=[]
                                ),
                                bass_nofuse=True,
                            )
                        )
                    ins.sync_info = mybir.SyncInfo(
                        on_wait=tail, on_update=list(si.on_update)
                    )
                    n_split += 1
                    changed = True
            new_insts.append(ins)
        if changed:
            bb.instructions = new_insts
    return n_split


def _elide_redundant_ldweights(nc):
    """Drop an InstLdweights that reloads the exact stationary loaded by the
    immediately preceding InstLdweights on PE (the score loop's q2 pairs),
    provided it carries no semaphore waits. The pinned walrus build runs
    with --enable-ldw-opt=false, so this elision does not happen downstream."""
    n = 0
    for bb in nc.main_func.blocks:
        insts = bb.instructions
        keep = []
        last_ldw_key = None
        for ins in insts:
            nm = type(ins).__name__
            eng = getattr(ins, "engine", None)
            if eng == mybir.EngineType.PE:
                if nm == "InstLdweights":
                    si = getattr(ins, "sync_info", None)
                    has_waits = si is not None and len(si.on_wait) > 0
                    has_updates = si is not None and len(si.on_update) > 0
                    key = repr(ins.ins)
                    if key == last_ldw_key:
                        if has_waits or has_updates:
                            # keep ordering semantics: park the waits and
                            # updates on a PE no-op in place of the load
                            ins2 = mybir.InstNoOp(
                                name=f"{ins.name}-ldwx",
                                engine=ins.engine,
                                sync_info=ins.sync_info,
                                bass_nofuse=True,
                            )
                            keep.append(ins2)
                        n += 1
                        continue  # elide the redundant weight load
                    last_ldw_key = key
                elif nm not in ("InstMatmult",):
                    # any other PE instruction invalidates the loaded weights
                    last_ldw_key = None
            if nm == "InstMatmult" and getattr(ins, "is_transpose", False):
                last_ldw_key = None
            keep.append(ins)
        bb.instructions = keep
    return n


def _dram_ap(handle, ap, offset=0):
    return bass.AP(
        tensor=handle.tensor if hasattr(handle, "tensor") else handle,
        offset=offset,
        ap=ap,
    )


def build_attention_nc(R=1):
    nc = bass.Bass()
    xT_e = nc.declare_dram_parameter("xT", [C, N], BF16, isOutput=False)
    wqT_e = nc.declare_dram_parameter("wqT", [C, DH], BF16, isOutput=False)
    wkT_e = nc.declare_dram_parameter("wkT", [C, DH], BF16, isOutput=False)
    wvT_e = nc.declare_dram_parameter("wvT", [C, DH], BF16, isOutput=False)
    wpT_e = nc.declare_dram_parameter("wpT", [DH, OUTC], BF16, isOutput=False)
    pb_e = nc.declare_dram_parameter("pb", [1, OUTC], BF16, isOutput=False)
    out_e = nc.declare_dram_parameter("out", [N, OUTC], BF16, isOutput=True)

    with tile.TileContext(nc) as tc:

        def body(_iv=None):
            with ExitStack() as ctx:
                persist = ctx.enter_context(tc.tile_pool(name="persist", bufs=1))
                XTb = persist.tile([128, CO, N], BF16)
                WQb = persist.tile([128, CO, DH], BF16)
                WKb = persist.tile([128, CO, DH], BF16)
                WVb = persist.tile([128, CO, DH], BF16)
                WPb = persist.tile([128, DO, OUTC], BF16)
                bias_sb = persist.tile([128, OUTC], BF16)
                KT = persist.tile([128, DO, N], BF16)
                QT = persist.tile([128, DO, N], BF16)
                VN = persist.tile([128, NO128, HH, HD + 1], BF16)
                OT = persist.tile([128, DO, N], BF16)
                ident = persist.tile([128, 128], BF16)
                make_identity(nc, ident)

                # ---- batched input DMAs (bf16) ----
                # weights first, then x in per-co chunks: the prelude's QKV
                # accumulation consumes chunks as they land instead of
                # stalling on one monolithic 4MB transfer
                # loads go out on the ScalarE HWDGE queue: its sequencer is
                # free at the iteration boundary (SP is still draining the
                # previous iteration's output-store queue), so next-iteration
                # input transfers overlap the previous tail
                for w_e, Wb in ((wqT_e, WQb), (wkT_e, WKb), (wvT_e, WVb)):
                    nc.scalar.dma_start(
                        out=Wb,
                        in_=_dram_ap(w_e, [[DH, 128], [DH * 128, CO], [1, DH]]),
                    )
                for co in range(CO):
                    nc.scalar.dma_start(
                        out=XTb[:, co, :],
                        in_=_dram_ap(
                            xT_e, [[N, 128], [1, N]], offset=co * 128 * N
                        ),
                    )
                nc.scalar.dma_start(
                    out=WPb,
                    in_=_dram_ap(wpT_e, [[OUTC, 128], [OUTC * 128, DO], [1, OUTC]]),
                )
                nc.scalar.dma_start(
                    out=bias_sb, in_=_dram_ap(pb_e, [[0, 128], [1, OUTC]])
                )

                with tc.tile_pool(name="psS", bufs=3, space="PSUM") as psS, \
                     tc.tile_pool(name="psO", bufs=2, space="PSUM") as psO, \
                     tc.tile_pool(name="asb", bufs=2) as asb:
                    nc.vector.memset(VN[:, :, :, HD], 1.0)

                    # ---------- side tasks ----------
                    def kq_task(Wb, DST, dslot, no):
                        def run():
                            ps = psS.tile([128, QB], F32, tag="psS", bufs=3)
                            seg = ps[:, 0:512]
                            for co in range(CO):
                                nc.tensor.matmul(
                                    seg,
                                    lhsT=Wb[:, co, dslot * 128 : (dslot + 1) * 128],
                                    rhs=XTb[:, co, no * 512 : (no + 1) * 512],
                                    start=(co == 0),
                                    stop=(co == CO - 1),
                                )
                            nc.vector.tensor_copy(
                                DST[:, dslot, no * 512 : (no + 1) * 512], seg
                            )

                        return run

                    def v_task(no):
                        def run():
                            ps = psS.tile([128, QB], F32, tag="psS", bufs=3)
                            seg = ps[:, 0:DH]
                            for co in range(CO):
                                nc.tensor.matmul(
                                    seg,
                                    lhsT=XTb[:, co, no * 128 : (no + 1) * 128],
                                    rhs=WVb[:, co, :],
                                    start=(co == 0),
                                    stop=(co == CO - 1),
                                )
                            nc.vector.tensor_copy(
                                VN[:, no, :, 0:HD],
                                seg.rearrange("p (h d) -> p h d", h=HH),
                            )

                        return run

                    def proj_task(no):
                        def run():
                            ps = psS.tile([128, QB], F32, tag="psS", bufs=3)
                            st = asb.tile([128, OUTC], BF16, tag="st", bufs=2)
                            for oc in range(2):
                                seg = ps[:, oc * 512 : (oc + 1) * 512]
                                for ci in range(DO):
                                    nc.tensor.matmul(
                                        seg,
                                        lhsT=OT[:, ci, no * 128 : (no + 1) * 128],
                                        rhs=WPb[:, ci, oc * 512 : (oc + 1) * 512],
                                        start=(ci == 0),
                                        stop=(ci == DO - 1),
                                    )
                            for oc in range(2):
                                nc.vector.tensor_tensor(
                                    st[:, oc * 512 : (oc + 1) * 512],
                                    ps[:, oc * 512 : (oc + 1) * 512],
                                    bias_sb[:, oc * 512 : (oc + 1) * 512],
                                    mybir.AluOpType.add,
                                )
                            nc.sync.dma_start(
                                out=out_e[no * 128 : (no + 1) * 128, :], in_=st
                            )

                        return run

                    # deferred-transpose slot: (ps_t, ob, OT-dst)
                    pend = [None]

                    def flush_tr():
                        if pend[0] is not None:
                            ps_t, ob, dst = pend[0]
                            nc.tensor.transpose(ps_t, ob, ident)
                            nc.vector.tensor_copy(dst, ps_t)
                            pend[0] = None

                    # zero-padded Q rings: scores run 128-deep (K pair
                    # stacked on partitions; the other head's Q rows are
                    # zero so its K contributes exactly 0). Built by the
                    # otherwise-idle Pool engine.
                    QZ_of = {}

                    def qz_task(u):
                        qb, h = divmod(u, HH)
                        row = (h % 2) * HD
                        dslot = h // 2

                        def run():
                            t = asb.tile(
                                [128, QB],
                                BF16,
                                tag="QZe" if h % 2 == 0 else "QZo",
                                bufs=2,
                                name=f"qz{u}",
                            )
                            dead = slice(HD, 128) if h % 2 == 0 else slice(0, HD)
                            nc.gpsimd.memset(t[dead, :], 0.0)
                            nc.gpsimd.tensor_copy(
                                t[row : row + HD, :],
                                QT[row : row + HD, dslot, qb * QB : (qb + 1) * QB],
                            )
                            QZ_of[u] = t

                        return run

                    def pv_task(ET, u, qo):
                        qb, h = divmod(u, HH)
                        row = (h % 2) * HD
                        dslot = h // 2

                        def run():
                            ps_ot = psO.tile([128, 512], F32, tag="psO")
                            ps_o = ps_ot[:, 0 : HD + 1]
                            ps_t = ps_ot.bitcast(BF16)[0:HD, 512 : 512 + 128]
                            for ko in range(KO):
                                nc.tensor.matmul(
                                    ps_o,
                                    lhsT=ET[:, ko, qo * 128 : (qo + 1) * 128],
                                    rhs=VN[:, ko, h, :],
                                    start=(ko == 0),
                                    stop=(ko == KO - 1),
                                )
                            flush_tr()
                            rcp = asb.tile([128, 1], F32, tag="rcp", bufs=3)
                            nc.vector.reciprocal(rcp, ps_o[:, HD : HD + 1])
                            ob = asb.tile([128, HD], BF16, tag="ob", bufs=4)
                            nc.vector.tensor_tensor(
                                ob,
                                ps_o[:, 0:HD],
                                rcp[:, 0:1].to_broadcast([128, HD]),
                                mybir.AluOpType.mult,
                            )
                            pend[0] = (
                                ps_t,
                                ob,
                                OT[
                                    row : row + HD,
                                    dslot,
                                    qb * QB + qo * 128 : qb * QB + (qo + 1) * 128,
                                ],
                            )

                        return run

                    # ---------- static side-task schedule ----------
                    # (earliest_chunk, closure) lists per unit; popped FIFO,
                    # up to 2 per score chunk once earliest_chunk is reached.
                    ET_of = {}

                    def unit_hdr(u):
                        ET = asb.tile([128, KO, QB], BF16, tag="ET", bufs=2)
                        ET_of[u] = ET
                        return ET

                    prelude = []
                    for no in range(NO512):
                        prelude.append(kq_task(WKb, KT, 0, no))
                    prelude.append(kq_task(WQb, QT, 0, 0))
                    prelude.append(kq_task(WQb, QT, 0, 1))
                    prelude.append(qz_task(0))
                    prelude.append(qz_task(1))

                    side = {u: [] for u in range(NU)}
                    for no in range(14):
                        side[0].append((0, v_task(no)))
                    side[1].append((0, v_task(14)))
                    side[1].append((0, v_task(15)))
                    for d in (1, 2, 3):
                        tgt = d  # units 1,2,4 -> KT/QT dslot d (qb0 halves)
                        ua = {1: 1, 2: 2, 3: 4}[d]
                        for no in range(NO512):
                            side[ua].append((0, kq_task(WKb, KT, d, no)))
                        side[ua].append((0, kq_task(WQb, QT, d, 0)))
                        side[ua].append((0, kq_task(WQb, QT, d, 1)))
                    for d in (0, 1, 2, 3):
                        ua = 6 + d  # qb1 halves of QT
                        side[ua].append((0, kq_task(WQb, QT, d, 2)))
                        side[ua].append((0, kq_task(WQb, QT, d, 3)))
                    for u in range(2, NU):
                        side[u - 1].append((0, qz_task(u)))
                    # PV of previous unit
                    for u in range(1, NU):
                        for qo in range(QBO):
                            side[u].append((2 + qo, None, u - 1, qo))
                    # output projection of qb block 0 during units 9-11
                    for j, no in enumerate(range(QBO)):
                        ua = 9 + j // 3
                        side[ua].append((3 + j % 3, proj_task(no)))

                    # ---------- emit the pipelined stream ----------
                    for t in prelude:
                        t()
                    for u in range(NU):
                        qb, h = divmod(u, HH)
                        row = (h % 2) * HD
                        dslot = h // 2
                        ET = unit_hdr(u)
                        ET16 = ET.bitcast(I16)
                        tasks = list(side[u])
                        ti = 0
                        QZ = QZ_of[u]
                        for ko in range(KO):
                            ps_s = psS.tile([128, QB], F32, tag="psS", bufs=3)
                            for q2 in range(QB // 512):
                                nc.tensor.matmul(
                                    ps_s[:, q2 * 512 : (q2 + 1) * 512],
                                    lhsT=KT[
                                        :, dslot, ko * 128 : (ko + 1) * 128
                                    ],
                                    rhs=QZ[:, q2 * 512 : (q2 + 1) * 512],
                                    start=True,
                                    stop=True,
                                )
                            if ko in DVE_KOS:
                                nc.vector.tensor_scalar(
                                    ET16[:, ko, :],
                                    ps_s,
                                    SCH_A,
                                    SCH_B,
                                    mybir.AluOpType.mult,
                                    mybir.AluOpType.add,
                                )
                            else:
                                nc.scalar.activation(
                                    out=ET[:, ko, :],
                                    in_=ps_s,
                                    func=mybir.ActivationFunctionType.Exp,
                                    scale=SCALE,
                                )
                            popped = 0
                            while ti < len(tasks) and popped < 2:
                                ent = tasks[ti]
                                if ent[0] > ko:
                                    break
                                if ent[1] is None:
                                    _, _, up, qo = ent
                                    pv_task(ET_of[up], up, qo)()
                                else:
                                    ent[1]()
                                ti += 1
                                popped += 1
                        # any leftovers (shouldn't happen, but be safe)
                        while ti < len(tasks):
                            ent = tasks[ti]
                            if ent[1] is None:
                                _, _, up, qo = ent
                                pv_task(ET_of[up], up, qo)()
                            else:
                                ent[1]()
                            ti += 1

                    # ---------- tail: last unit's PV + last block's proj ----
                    for qo in range(QBO):
                        pv_task(ET_of[NU - 1], NU - 1, qo)()
                    flush_tr()
                    for no in range(QBO, 2 * QBO):
                        proj_task(no)()

        if R == 1:
            body()
        else:
            with tc.For_i(0, R, 1, staggered_reset=True) as iv:
                body(iv)

    _elide_redundant_ldweights(nc)
    _split_multi_waits(nc)
    return nc


def shard_inputs(x, qkv_w, proj_w, proj_b):
    bf = ml_dtypes.bfloat16
    in_maps = []
    for i in range(8):
        b, g = i // 2, i % 2
        sl = slice(g * DH, (g + 1) * DH)
        xT = np.ascontiguousarray(np.asarray(x, np.float32)[b].T).astype(bf)
        wqT = np.ascontiguousarray(
            np.asarray(qkv_w, np.float32)[0 * C : 1 * C][sl, :].T
        ).astype(bf)
        wkT = np.ascontiguousarray(
            np.asarray(qkv_w, np.float32)[1 * C : 2 * C][sl, :].T
        ).astype(bf)
        wvT = np.ascontiguousarray(
            np.asarray(qkv_w, np.float32)[2 * C : 3 * C][sl, :].T
        ).astype(bf)
        wpT = np.ascontiguousarray(np.asarray(proj_w, np.float32)[:, sl].T).astype(bf)
        pb = (
            np.asarray(proj_b, np.float32)
            if g == 0
            else np.zeros_like(np.asarray(proj_b, np.float32))
        ).reshape(1, -1)
        in_maps.append(
            {
                "xT": xT,
                "wqT": wqT,
                "wkT": wkT,
                "wvT": wvT,
                "wpT": wpT,
                "pb": np.ascontiguousarray(pb).astype(bf),
            }
        )
    return in_maps


_CACHED_NC = None


def kernel(x, qkv_w, proj_w, proj_b):
    """Full inputs in, full output out. Shards over 8 NeuronCores."""
    global _CACHED_NC
    x = np.asarray(x, dtype=np.float32)
    qkv_w = np.asarray(qkv_w, dtype=np.float32)
    proj_w = np.asarray(proj_w, dtype=np.float32)
    proj_b = np.asarray(proj_b, dtype=np.float32)

    if _CACHED_NC is None:
        _CACHED_NC = build_attention_nc(R=1)
    nc = _CACHED_NC

    in_maps = shard_inputs(x, qkv_w, proj_w, proj_b)

    def run_once():
        res = run_bass_kernel_spmd(nc, in_maps, core_ids=list(range(8)))
        out = np.empty((B, N, OUTC), dtype=np.float32)
        for b in range(B):
            out[b] = res.results[2 * b]["out"].astype(np.float32) + res.results[
                2 * b + 1
            ]["out"].astype(np.float32)
        return out

    # The kernel is deterministic; rarely the relay/device glitches a run.
    # Run twice and accept on agreement, retrying the pair once if needed.
    prev = run_once()
    for _ in range(3):
        cur = run_once()
        if np.array_equal(prev, cur):
            return cur
        prev = cur
    return cur


# revision 28
# speedup vs baseline: 1.0039x; 1.0039x over previous
"""Multi-head attention (B=4, N=2048, C=1024, H=16, HD=64) on 8 TRN2 NeuronCores.

Self-contained: takes the FULL unsharded inputs of the reference problem,
shards across 8 cores, runs a Bass/Tile kernel SPMD, and reassembles the
full output.

Sharding (tensor-parallel over heads x data-parallel over batch):
  core i -> batch b = i//2, head-group g = i%2 (8 of the 16 heads).
  Unsharding sums the two partial output projections per batch.

Inputs are cast to bf16 on the host; partial outputs return as bf16 and
are summed in f32 on host.

The kernel is one long software-pipelined stream built around the
attention score chunks. Between score chunks the emitter interleaves
"side tasks" on the PE: QKV-projection tiles (just before the head that
needs them), V-projection tiles, the PV (attention*V) accumulations of
the previous head with their normalize/transpose epilogues, and
output-projection tiles of the previous query block. PE is the
bottleneck engine (~measured-rate roofline); ScalarE streams exp
underneath it.

Score matmuls run with a FULL 128-deep contraction even though HD=64:
the K of a head PAIR is stacked on partitions (natural layout), and the
moving Q operand is zero-padded on the other head's 64 rows, so the
other head contributes exactly 0. (64-deep matmuls stream at half rate
on TRN2 hardware; this trick doubles score throughput at zero accuracy
cost.) The padded per-head Q tiles are built into a small ring by the
otherwise-idle Pool engine.

exp split: 14/16 key-chunks on ScalarE (exact, with the softmax
denominator accumulated via an extra ones-column in V), 2/16 on DVE via
a Schraudolph-style fast exp (single tensor_scalar: bf16 bits =
round(s*a+b) as int16; zero-mean calibrated, ~1.8% elementwise noise
that averages out in the PV reduction).

PSUM: a 3-deep ring of [128,1024]f32 slots shared by score chunks,
QKV/V-projection tiles and output-projection tiles (6 banks), plus a
2-deep ring of single-bank slots each holding a PV accumulator (f32
cols 0:65) and the PE-transpose staging area (bf16 cols 512:640).
"""

import sys

sys.path.insert(0, "/opt/trn_rl_repo")

from contextlib import ExitStack

import numpy as np
import ml_dtypes

import concourse.bass as bass
import concourse.tile as tile
from concourse import mybir
from concourse.masks import make_identity
from concourse.bass_utils import run_bass_kernel_spmd

F32 = mybir.dt.float32
BF16 = mybir.dt.bfloat16
I16 = mybir.dt.int16

B, N, C, H = 4, 2048, 1024, 16
HH = H // 2          # heads per core
HD = C // H          # head dim
DH = HH * HD         # attention channels per core
OUTC = C
CO = C // 128
DO = DH // 128
NO128 = N // 128
NO512 = N // 512
KO = N // 128
QB = 1024
NQB = N // QB
QBO = QB // 128
NU = NQB * HH        # pipeline units
SCALE = float(HD) ** -0.5

LOG2E = 1.4426950408889634
SCH_C = 0.0581        # zero-mean Schraudolph bias (round-to-nearest convert)
SCH_A = SCALE * 128.0 * LOG2E
SCH_B = (127.0 - SCH_C) * 128.0
DVE_KOS = (5, 13)     # 2/16 key-chunks fast-exp on DVE: keeps ScalarE
                      # clearly below PE so the psS ring never throttles


def _split_multi_waits(nc, max_waits=1):
    """The pinned walrus build rejects >1 SyncWait on engine CTRL
    instructions; move extra waits onto preceding NOPs."""
    n_split = 0
    for bb in nc.main_func.blocks:
        insts = list(bb.instructions)
        new_insts = []
        changed = False
        for ins in insts:
            si = getattr(ins, "sync_info", None)
            nm = type(ins).__name__
            is_dma = "Dma" in nm or "TensorLoad" in nm or "TensorSave" in nm
            if si is not None and not is_dma:
                waits = list(si.on_wait)
                if len(waits) > max_waits:
                    head, tail = waits[:-max_waits], waits[-max_waits:]
                    for i in range(0, len(head), max_waits):
                        new_insts.append(
                            mybir.InstNoOp(
                                name=f"{ins.name}-ws{n_split}-{i}",
                                engine=ins.engine,
                                sync_info=mybir.SyncInfo(
                                    on_wait=head[i : i + max_waits], on_update=[]
                                ),
                                bass_nofuse=True,
                            )
                        )
                    ins.sync_info = mybir.SyncInfo(
                        on_wait=tail, on_update=list(si.on_update)
                    )
                    n_split += 1
                    changed = True
            new_insts.append(ins)
        if changed:
            bb.instructions = new_insts
    return n_split


def _elide_redundant_ldweights(nc):
    """Drop an InstLdweights that reloads the exact stationary loaded by the
    immediately preceding InstLdweights on PE (the score loop's q2 pairs),
    provided it carries no semaphore waits. The pinned walrus build runs
    with --enable-ldw-opt=false, so this elision does not happen downstream."""
    n = 0
    for bb in nc.main_func.blocks:
        insts = bb.instructions
        keep = []
        last_ldw_key = None
        for ins in insts:
            nm = type(ins).__name__
            eng = getattr(ins, "engine", None)
            if eng == mybir.EngineType.PE:
                if nm == "InstLdweights":
                    si = getattr(ins, "sync_info", None)
                    has_waits = si is not None and len(si.on_wait) > 0
                    has_updates = si is not None and len(si.on_update) > 0
                    key = repr(ins.ins)
                    if key == last_ldw_key:
                        if has_waits or has_updates:
                            # keep ordering semantics: park the waits and
                            # updates on a PE no-op in place of the load
                            ins2 = mybir.InstNoOp(
                                name=f"{ins.name}-ldwx",
                                engine=ins.engine,
                                sync_info=ins.sync_info,
                                bass_nofuse=True,
                            )
                            keep.append(ins2)
                        n += 1
                        continue  # elide the redundant weight load
                    last_ldw_key = key
                elif nm not in ("InstMatmult",):
                    # any other PE instruction invalidates the loaded weights
                    last_ldw_key = None
            if nm == "InstMatmult" and getattr(ins, "is_transpose", False):
                last_ldw_key = None
            keep.append(ins)
        bb.instructions = keep
    return n


def _dram_ap(handle, ap, offset=0):
    return bass.AP(
        tensor=handle.tensor if hasattr(handle, "tensor") else handle,
        offset=offset,
        ap=ap,
    )


def build_attention_nc(R=1):
    nc = bass.Bass()
    xT_e = nc.declare_dram_parameter("xT", [C, N], BF16, isOutput=False)
    wqT_e = nc.declare_dram_parameter("wqT", [C, DH], BF16, isOutput=False)
    wkT_e = nc.declare_dram_parameter("wkT", [C, DH], BF16, isOutput=False)
    wvT_e = nc.declare_dram_parameter("wvT", [C, DH], BF16, isOutput=False)
    wpT_e = nc.declare_dram_parameter("wpT", [DH, OUTC], BF16, isOutput=False)
    pb_e = nc.declare_dram_parameter("pb", [1, OUTC], BF16, isOutput=False)
    out_e = nc.declare_dram_parameter("out", [N, OUTC], BF16, isOutput=True)

    with tile.TileContext(nc) as tc:

        def body(_iv=None):
            with ExitStack() as ctx:
                persist = ctx.enter_context(tc.tile_pool(name="persist", bufs=1))
                XTb = persist.tile([128, CO, N], BF16)
                WQb = persist.tile([128, CO, DH], BF16)
                WKb = persist.tile([128, CO, DH], BF16)
                WVb = persist.tile([128, CO, DH], BF16)
                WPb = persist.tile([128, DO, OUTC], BF16)
                bias_sb = persist.tile([128, OUTC], BF16)
                KT = persist.tile([128, DO, N], BF16)
                QT = persist.tile([128, DO, N], BF16)
                VN = persist.tile([128, NO128, HH, HD + 1], BF16)
                OT = persist.tile([128, DO, N], BF16)
                ident = persist.tile([128, 128], BF16)
                make_identity(nc, ident)

                # ---- batched input DMAs (bf16) ----
                # weights first, then x in per-co chunks: the prelude's QKV
                # accumulation consumes chunks as they land instead of
                # stalling on one monolithic 4MB transfer
                for w_e, Wb in ((wqT_e, WQb), (wkT_e, WKb), (wvT_e, WVb)):
                    nc.sync.dma_start(
                        out=Wb,
                        in_=_dram_ap(w_e, [[DH, 128], [DH * 128, CO], [1, DH]]),
                    )
                for co in range(CO):
                    nc.sync.dma_start(
                        out=XTb[:, co, :],
                        in_=_dram_ap(
                            xT_e, [[N, 128], [1, N]], offset=co * 128 * N
                        ),
                    )
                nc.sync.dma_start(
                    out=WPb,
                    in_=_dram_ap(wpT_e, [[OUTC, 128], [OUTC * 128, DO], [1, OUTC]]),
                )
                nc.sync.dma_start(
                    out=bias_sb, in_=_dram_ap(pb_e, [[0, 128], [1, OUTC]])
                )

                with tc.tile_pool(name="psS", bufs=3, space="PSUM") as psS, \
                     tc.tile_pool(name="psO", bufs=2, space="PSUM") as psO, \
                     tc.tile_pool(name="asb", bufs=2) as asb:
                    nc.vector.memset(VN[:, :, :, HD], 1.0)

                    # ---------- side tasks ----------
                    def kq_task(Wb, DST, dslot, no):
                        def run():
                            ps = psS.tile([128, QB], F32, tag="psS", bufs=3)
                            seg = ps[:, 0:512]
                            for co in range(CO):
                                nc.tensor.matmul(
                                    seg,
                                    lhsT=Wb[:, co, dslot * 128 : (dslot + 1) * 128],
                                    rhs=XTb[:, co, no * 512 : (no + 1) * 512],
                                    start=(co == 0),
                                    stop=(co == CO - 1),
                                )
                            nc.vector.tensor_copy(
                                DST[:, dslot, no * 512 : (no + 1) * 512], seg
                            )

                        return run

                    def v_task(no):
                        def run():
                            ps = psS.tile([128, QB], F32, tag="psS", bufs=3)
                            seg = ps[:, 0:DH]
                            for co in range(CO):
                                nc.tensor.matmul(
                                    seg,
                                    lhsT=XTb[:, co, no * 128 : (no + 1) * 128],
                                    rhs=WVb[:, co, :],
                                    start=(co == 0),
                                    stop=(co == CO - 1),
                                )
                            nc.vector.tensor_copy(
                                VN[:, no, :, 0:HD],
                                seg.rearrange("p (h d) -> p h d", h=HH),
                            )

                        return run

                    def proj_task(no):
                        def run():
                            ps = psS.tile([128, QB], F32, tag="psS", bufs=3)
                            st = asb.tile([128, OUTC], BF16, tag="st", bufs=2)
                            for oc in range(2):
                                seg = ps[:, oc * 512 : (oc + 1) * 512]
                                for ci in range(DO):
                                    nc.tensor.matmul(
                                        seg,
                                        lhsT=OT[:, ci, no * 128 : (no + 1) * 128],
                                        rhs=WPb[:, ci, oc * 512 : (oc + 1) * 512],
                                        start=(ci == 0),
                                        stop=(ci == DO - 1),
                                    )
                            for oc in range(2):
                                nc.vector.tensor_tensor(
                                    st[:, oc * 512 : (oc + 1) * 512],
                                    ps[:, oc * 512 : (oc + 1) * 512],
                                    bias_sb[:, oc * 512 : (oc + 1) * 512],
                                    mybir.AluOpType.add,
                                )
                            nc.sync.dma_start(
                                out=out_e[no * 128 : (no + 1) * 128, :], in_=st
                            )

                        return run

                    # deferred-transpose slot: (ps_t, ob, OT-dst)
                    pend = [None]

                    def flush_tr():
                        if pend[0] is not None:
                            ps_t, ob, dst = pend[0]
                            nc.tensor.transpose(ps_t, ob, ident)
                            nc.vector.tensor_copy(dst, ps_t)
                            pend[0] = None

                    # zero-padded Q rings: scores run 128-deep (K pair
                    # stacked on partitions; the other head's Q rows are
                    # zero so its K contributes exactly 0). Built by the
                    # otherwise-idle Pool engine.
                    QZ_of = {}

                    def qz_task(u):
                        qb, h = divmod(u, HH)
                        row = (h % 2) * HD
                        dslot = h // 2

                        def run():
                            t = asb.tile(
                                [128, QB],
                                BF16,
                                tag="QZe" if h % 2 == 0 else "QZo",
                                bufs=2,
                                name=f"qz{u}",
                            )
                            dead = slice(HD, 128) if h % 2 == 0 else slice(0, HD)
                            nc.gpsimd.memset(t[dead, :], 0.0)
                            nc.gpsimd.tensor_copy(
                                t[row : row + HD, :],
                                QT[row : row + HD, dslot, qb * QB : (qb + 1) * QB],
                            )
                            QZ_of[u] = t

                        return run

                    def pv_task(ET, u, qo):
                        qb, h = divmod(u, HH)
                        row = (h % 2) * HD
                        dslot = h // 2

                        def run():
                            ps_ot = psO.tile([128, 512], F32, tag="psO")
                            ps_o = ps_ot[:, 0 : HD + 1]
                            ps_t = ps_ot.bitcast(BF16)[0:HD, 512 : 512 + 128]
                            for ko in range(KO):
                                nc.tensor.matmul(
                                    ps_o,
                                    lhsT=ET[:, ko, qo * 128 : (qo + 1) * 128],
                                    rhs=VN[:, ko, h, :],
                                    start=(ko == 0),
                                    stop=(ko == KO - 1),
                                )
                            flush_tr()
                            rcp = asb.tile([128, 1], F32, tag="rcp", bufs=3)
                            nc.vector.reciprocal(rcp, ps_o[:, HD : HD + 1])
                            ob = asb.tile([128, HD], BF16, tag="ob", bufs=4)
                            nc.vector.tensor_tensor(
                                ob,
                                ps_o[:, 0:HD],
                                rcp[:, 0:1].to_broadcast([128, HD]),
                                mybir.AluOpType.mult,
                            )
                            pend[0] = (
                                ps_t,
                                ob,
                                OT[
                                    row : row + HD,
                                    dslot,
                                    qb * QB + qo * 128 : qb * QB + (qo + 1) * 128,
                                ],
                            )

                        return run

                    # ---------- static side-task schedule ----------
                    # (earliest_chunk, closure) lists per unit; popped FIFO,
                    # up to 2 per score chunk once earliest_chunk is reached.
                    ET_of = {}

                    def unit_hdr(u):
                        ET = asb.tile([128, KO, QB], BF16, tag="ET", bufs=2)
                        ET_of[u] = ET
                        return ET

                    prelude = []
                    for no in range(NO512):
                        prelude.append(kq_task(WKb, KT, 0, no))
                    prelude.append(kq_task(WQb, QT, 0, 0))
                    prelude.append(kq_task(WQb, QT, 0, 1))
                    prelude.append(qz_task(0))
                    prelude.append(qz_task(1))

                    side = {u: [] for u in range(NU)}
                    for no in range(14):
                        side[0].append((0, v_task(no)))
                    side[1].append((0, v_task(14)))
                    side[1].append((0, v_task(15)))
                    for d in (1, 2, 3):
                        tgt = d  # units 1,2,4 -> KT/QT dslot d (qb0 halves)
                        ua = {1: 1, 2: 2, 3: 4}[d]
                        for no in range(NO512):
                            side[ua].append((0, kq_task(WKb, KT, d, no)))
                        side[ua].append((0, kq_task(WQb, QT, d, 0)))
                        side[ua].append((0, kq_task(WQb, QT, d, 1)))
                    for d in (0, 1, 2, 3):
                        ua = 6 + d  # qb1 halves of QT
                        side[ua].append((0, kq_task(WQb, QT, d, 2)))
                        side[ua].append((0, kq_task(WQb, QT, d, 3)))
                    for u in range(2, NU):
                        side[u - 1].append((0, qz_task(u)))
                    # PV of previous unit
                    for u in range(1, NU):
                        for qo in range(QBO):
                            side[u].append((2 + qo, None, u - 1, qo))
                    # output projection of qb block 0 during units 9-11
                    for j, no in enumerate(range(QBO)):
                        ua = 9 + j // 3
                        side[ua].append((3 + j % 3, proj_task(no)))

                    # ---------- emit the pipelined stream ----------
                    for t in prelude:
                        t()
                    for u in range(NU):
                        qb, h = divmod(u, HH)
                        row = (h % 2) * HD
                        dslot = h // 2
                        ET = unit_hdr(u)
                        ET16 = ET.bitcast(I16)
                        tasks = list(side[u])
                        ti = 0
                        QZ = QZ_of[u]
                        for ko in range(KO):
                            ps_s = psS.tile([128, QB], F32, tag="psS", bufs=3)
                            for q2 in range(QB // 512):
                                nc.tensor.matmul(
                                    ps_s[:, q2 * 512 : (q2 + 1) * 512],
                                    lhsT=KT[
                                        :, dslot, ko * 128 : (ko + 1) * 128
                                    ],
                                    rhs=QZ[:, q2 * 512 : (q2 + 1) * 512],
                                    start=True,
                                    stop=True,
                                )
                            if ko in DVE_KOS:
                                nc.vector.tensor_scalar(
                                    ET16[:, ko, :],
                                    ps_s,
                                    SCH_A,
                                    SCH_B,
                                    mybir.AluOpType.mult,
                                    mybir.AluOpType.add,
                                )
                            else:
                                nc.scalar.activation(
                                    out=ET[:, ko, :],
                                    in_=ps_s,
                                    func=mybir.ActivationFunctionType.Exp,
                                    scale=SCALE,
                                )
                            popped = 0
                            while ti < len(tasks) and popped < 2:
                                ent = tasks[ti]
                                if ent[0] > ko:
                                    break
                                if ent[1] is None:
                                    _, _, up, qo = ent
                                    pv_task(ET_of[up], up, qo)()
                                else:
                                    ent[1]()
                                ti += 1
                                popped += 1
                        # any leftovers (shouldn't happen, but be safe)
                        while ti < len(tasks):
                            ent = tasks[ti]
                            if ent[1] is None:
                                _, _, up, qo = ent
                                pv_task(ET_of[up], up, qo)()
                            else:
                                ent[1]()
                            ti += 1

                    # ---------- tail: last unit's PV + last block's proj ----
                    for qo in range(QBO):
                        pv_task(ET_of[NU - 1], NU - 1, qo)()
                    flush_tr()
                    for no in range(QBO, 2 * QBO):
                        proj_task(no)()

        if R == 1:
            body()
        else:
            with tc.For_i(
                0, R, 1, staggered_reset=True,
                hint_engines=tuple(mybir.ALL_ENGINES),
            ) as iv:
                body(iv)

    _elide_redundant_ldweights(nc)
    _split_multi_waits(nc)
    return nc


def shard_inputs(x, qkv_w, proj_w, proj_b):
    bf = ml_dtypes.bfloat16
    in_maps = []
    for i in range(8):
        b, g = i // 2, i % 2
        sl = slice(g * DH, (g + 1) * DH)
        xT = np.ascontiguousarray(np.asarray(x, np.float32)[b].T).astype(bf)
        wqT = np.ascontiguousarray(
            np.asarray(qkv_w, np.float32)[0 * C : 1 * C][sl, :].T
        ).astype(bf)
        wkT = np.ascontiguousarray(
            np.asarray(qkv_w, np.float32)[1 * C : 2 * C][sl, :].T
        ).astype(bf)
        wvT = np.ascontiguousarray(
            np.asarray(qkv_w, np.float32)[2 * C : 3 * C][sl, :].T
        ).astype(bf)
        wpT = np.ascontiguousarray(np.asarray(proj_w, np.float32)[:, sl].T).astype(bf)
        pb = (
            np.asarray(proj_b, np.float32)
            if g == 0
            else np.zeros_like(np.asarray(proj_b, np.float32))
        ).reshape(1, -1)
        in_maps.append(
            {
                "xT": xT,
                "wqT": wqT,
                "wkT": wkT,
                "wvT": wvT,
                "wpT": wpT,
                "pb": np.ascontiguousarray(pb).astype(bf),
            }
        )
    return in_maps


_CACHED_NC = None


def kernel(x, qkv_w, proj_w, proj_b):
    """Full inputs in, full output out. Shards over 8 NeuronCores."""
    global _CACHED_NC
    x = np.asarray(x, dtype=np.float32)
    qkv_w = np.asarray(qkv_w, dtype=np.float32)
    proj_w = np.asarray(proj_w, dtype=np.float32)
    proj_b = np.asarray(proj_b, dtype=np.float32)

    if _CACHED_NC is None:
        _CACHED_NC = build_attention_nc(R=1)
    nc = _CACHED_NC

    in_maps = shard_inputs(x, qkv_w, proj_w, proj_b)

    def run_once():
        res = run_bass_kernel_spmd(nc, in_maps, core_ids=list(range(8)))
        out = np.empty((B, N, OUTC), dtype=np.float32)
        for b in range(B):
            out[b] = res.results[2 * b]["out"].astype(np.float32) + res.results[
                2 * b + 1
            ]["out"].astype(np.float32)
        return out

    # The kernel is deterministic; rarely the relay/device glitches a run.
    # Run twice and accept on agreement, retrying the pair once if needed.
    prev = run_once()
    for _ in range(3):
        cur = run_once()
        if np.array_equal(prev, cur):
            return cur
        prev = cur
    return cur
